# revision 28
# baseline (speedup 1.0000x reference)
"""Paged-attention decode kernel for 8 TRN2 NeuronCores.

Sharding: tensor-parallel over the 8 KV heads (one per core). Each core holds
its own 128-wide slice of the paged KV cache (converted to bf16), computes the
4 GQA query heads of its group for all 32 requests, and writes a [128, 128]
output block ([32 req x 4 heads, 128 dim]). The host applies the KV-cache
scatter update, builds per-core pools/indices/masks, and concatenates the 8
per-core outputs into the full [32, 32, 128] result. No collectives needed.

Device algorithm per core:
  - dma_gather(transpose=True) over block rows of the bf16 K pool yields the
    K^T layout [d=128, slot, block] directly (one gather per 4-request group).
  - QK matmuls use a zero-padded stationary q so request b's scores land on
    PSUM partitions 4b..4b+3; all 32 requests accumulate into one
    [128, 2048] PSUM scores region ([req*4+head, slot*128+block]).
  - Batched masked softmax over the full [128, 2048] region (mask from
    context_lens, built host-side).
  - 16 PE transposes produce p^T; dma_gather(transpose=False) yields V blocks
    [block, slot*128+d]; PV matmuls contract over blocks per (request, slot),
    accumulating [4, 128] per request in PSUM.
"""

import os
import sys

import numpy as np
import ml_dtypes

if "/opt/trn_rl_repo" not in sys.path:
    sys.path.insert(0, "/opt/trn_rl_repo")

import concourse.bacc as bacc
import concourse.bass as bass
import concourse.mybir as mybir
import concourse.tile as tile

BF16 = ml_dtypes.bfloat16

SCALE = 0.08838834764831845  # 1/sqrt(128)
B = 32               # requests
KVH = 8              # kv heads == cores
NH = 4               # q heads per kv head (GQA group)
DH = 128             # head dim
BS = 16              # tokens per cache block
NBLOCKS = 4096       # pool blocks
MBS = 128            # max blocks per sequence
S = MBS * BS         # 2048 max context
GROUPS = 16          # request groups per core (pairs: large ctx + small ctx)
GR = B // GROUPS     # 2 requests per group
NIDX = GR * MBS      # 256 gathered blocks per group
NEG = -1.0e30


NQUEUES = 1
DETECT_RACES = True  # sim-only; the SWDGE-prep sem rewrite confuses the
                     # race detector's semaphore epoch accounting


def build_core_program():
    """Build the single-core Bass program (same on all 8 cores)."""
    nc = bacc.Bacc(
        "TRN2", target_bir_lowering=False, num_swdge_queues=NQUEUES,
        detect_race_conditions=DETECT_RACES,
    )
    f32 = mybir.dt.float32
    bf16 = mybir.dt.bfloat16
    i16 = mybir.dt.int16

    k_pool = nc.dram_tensor("k_pool", [NBLOCKS, BS * DH], bf16, kind="ExternalInput")
    v_pool = nc.dram_tensor("v_pool", [NBLOCKS, BS * DH], bf16, kind="ExternalInput")
    qpad = nc.dram_tensor("qpad", [DH, B * 128], bf16, kind="ExternalInput")
    maskd = nc.dram_tensor("mask", [128, S], f32, kind="ExternalInput")
    idxd = nc.dram_tensor("idx", [128, GROUPS * (NIDX // 16)], i16, kind="ExternalInput")
    nvd = nc.dram_tensor("nv", [1, GROUPS], mybir.dt.int32, kind="ExternalInput")
    ident = nc.dram_tensor("ident", [128, 128], bf16, kind="ExternalInput")
    out = nc.dram_tensor("out", [128, DH], f32, kind="ExternalOutput")

    Exp = mybir.ActivationFunctionType.Exp
    ICOLS = NIDX // 16  # 32 idx columns per group

    with tile.TileContext(nc) as tc:
        with (
            tc.tile_pool(name="const", bufs=1) as cpool,
            tc.tile_pool(name="soft", bufs=1) as spool,
            tc.tile_pool(name="kt", bufs=3) as ktpool,
            tc.tile_pool(name="vv", bufs=3) as vpool,
            tc.tile_pool(name="outs", bufs=4) as ospool,
        ):
            # preload the gather ucode library so its ~13us fetch overlaps
            # the input DMAs instead of stalling the first gather
            from concourse.library_config import mlp as _mlp_lib
            nc.gpsimd.load_library(_mlp_lib)

            qpad_sb = cpool.tile([DH, B * 128], bf16)
            mask_sb = cpool.tile([128, S], f32)
            idx_sb = cpool.tile([128, GROUPS * ICOLS], i16)
            nv_sb = cpool.tile([1, GROUPS], mybir.dt.int32)
            id_sb = cpool.tile([128, 128], bf16)
            nc.sync.dma_start(idx_sb[:], idxd[:])
            nc.sync.dma_start(nv_sb[:], nvd[:])
            nc.sync.dma_start(qpad_sb[:], qpad[:])
            nc.sync.dma_start(mask_sb[:], maskd[:])
            nc.sync.dma_start(id_sb[:], ident[:])

            # per-group valid-idx counts in persistent registers (a fresh
            # to_reg per gather would add a MOVE whose WAR dep serializes
            # gathers on the previous gather's DMA completion)
            nv_regs = []
            for g in range(GROUPS):
                r = nc.gpsimd.alloc_register(f"nv{g}")
                nc.gpsimd.reg_load(r, nv_sb[0:1, g:g + 1])
                nv_regs.append(r)

            s_sb = spool.tile([128, S], f32)
            p_sb = spool.tile([128, S], bf16)
            p2_sb = spool.tile([128, S], bf16)
            pt_sb = spool.tile([128, S], bf16)
            mx = spool.tile([128, 1], f32)
            negm = spool.tile([128, 1], f32)
            sums = spool.tile([128, 1], f32)
            recip = spool.tile([128, 1], f32)

            # ---- Phase B: K gathers + QK matmuls into one PSUM scores region
            NMM = (BS + 3) // 4  # N<=512 chunks of up-to-4 slots each
            with tc.tile_pool(name="pscore", bufs=1, space="PSUM") as pspool:
                scores = pspool.tile([128, S], f32)
                for g in range(GROUPS):
                    kt = ktpool.tile([128, BS, NIDX], bf16, tag="kt")
                    # clear before gather: the valid-skip tail is never
                    # written, and uninitialized SBUF can hold NaN patterns
                    # that would poison the masked softmax
                    nc.vector.memset(kt[:], 0)
                    nc.gpsimd.dma_gather(
                        kt[:],
                        k_pool[:],
                        idx_sb[:, g * ICOLS:(g + 1) * ICOLS],
                        NIDX,
                        nv_regs[g],
                        BS * DH,
                        transpose=True,
                        queue_num=g % NQUEUES,
                    )
                    for r in range(GR):
                        b = GR * g + r
                        for mm in range(NMM):
                            nsl = min(4, BS - mm * 4)
                            nc.tensor.matmul(
                                scores[:, mm * 512: mm * 512 + nsl * 128],
                                lhsT=qpad_sb[:, b * 128:(b + 1) * 128],
                                rhs=kt[:, mm * 4: mm * 4 + nsl, r * 128:(r + 1) * 128],
                                start=(b == 0),
                                stop=(b == B - 1),
                            )

                # ---- Phase C: batched masked softmax
                nc.vector.tensor_tensor(
                    out=s_sb[:], in0=scores[:], in1=mask_sb[:], op=mybir.AluOpType.add
                )
            nc.vector.reduce_max(mx[:], s_sb[:], axis=mybir.AxisListType.X)
            nc.scalar.mul(negm[:], mx[:], -1.0)
            nc.scalar.activation(
                p_sb[:], s_sb[:], Exp, bias=negm[:, 0:1], scale=1.0,
                accum_out=sums[:, 0:1],
            )
            nc.vector.reciprocal(recip[:], sums[:])
            nc.vector.tensor_scalar_mul(p2_sb[:], p_sb[:], recip[:, 0:1])

            # ---- Phase D: p^T via PE transposes
            with tc.tile_pool(name="ptr", bufs=2, space="PSUM") as tppool:
                for cc in range(BS):
                    tp = tppool.tile([128, 128], bf16, tag="tp")
                    nc.tensor.transpose(tp[:], p2_sb[:, cc * 128:(cc + 1) * 128], id_sb[:])
                    if cc % 2 == 0:
                        nc.vector.tensor_copy(pt_sb[:, cc * 128:(cc + 1) * 128], tp[:])
                    else:
                        nc.scalar.copy(pt_sb[:, cc * 128:(cc + 1) * 128], tp[:])

            # ---- Phase E: V gathers + PV matmuls
            with tc.tile_pool(name="pout", bufs=4, space="PSUM") as popool:
                for g in range(GROUPS):
                    vt = vpool.tile([128, GR, BS * DH], bf16, tag="vt")
                    nc.vector.memset(vt[:], 0)
                    nc.gpsimd.dma_gather(
                        vt[:],
                        v_pool[:],
                        idx_sb[:, g * ICOLS:(g + 1) * ICOLS],
                        NIDX,
                        nv_regs[g],
                        BS * DH,
                        transpose=False,
                        queue_num=g % NQUEUES,
                    )
                    for r in range(GR):
                        b = GR * g + r
                        po = popool.tile([NH, DH], mybir.dt.float32, tag="po")
                        for sl in range(BS):
                            nc.tensor.matmul(
                                po[:],
                                lhsT=pt_sb[:, sl * 128 + NH * b: sl * 128 + NH * b + NH],
                                rhs=vt[:, r, sl * DH:(sl + 1) * DH],
                                start=(sl == 0),
                                stop=(sl == BS - 1),
                            )
                        os_t = ospool.tile([NH, DH], mybir.dt.float32, tag="os")
                        nc.vector.tensor_copy(os_t[:], po[:])
                        nc.sync.dma_start(out[NH * b: NH * b + NH, :], os_t[:])

    nc.compile()
    _fix_prep_completion_sems(nc)
    return nc


def _fix_prep_completion_sems(nc):
    """Tile gates consumers of a prepare_only SWDGE gather on its DMASW lane
    semaphore, but the DMA-completion sem baked into the descriptors stays the
    caller-provided one — the lane sem would never fire. Rewrite each prep's
    on_update[0] to the lane sem of its scheduled DMASW proc."""
    from concourse.tile_sem_assignment import PROC_NAME_TO_IDX

    idx_to_lane = {v: k for k, v in PROC_NAME_TO_IDX.items() if "DMASW" in k}
    # sem ant_name -> (id, name) for tile-created DMASW sems
    sems = {}
    for bb in nc.main_func.blocks:
        for ins in bb.instructions:
            si = ins.sync_info
            if not si:
                continue
            for ev in list(si.on_wait or []) + list(si.on_update or []):
                name = getattr(ev, "ant_name", None)
                if name and name.startswith("DMASW"):
                    sems[name.split("_")[0]] = (ev.id, name)
    for bb in nc.main_func.blocks:
        for ins in bb.instructions:
            if type(ins).__name__ != "InstDMAGatherAnt" or ins.gen_mode != 1:
                continue
            proc = ins.bass_scheduled_proc
            lane = idx_to_lane.get(proc)
            assert lane is not None, f"prep {ins.name} not on a DMASW lane: {proc}"
            assert lane in sems, f"no tile sem found for {lane}"
            sid, sname = sems[lane]
            upd = ins.sync_info.on_update[0]
            assert upd.ant_name.startswith("kdma"), upd.ant_name
            upd.id = sid
            upd.ant_name = sname


def _host_inputs(q, k, v, k_cache, v_cache, slot_mapping, block_tables, context_lens):
    """Apply the scatter update and build per-core input dicts."""
    D = KVH * DH
    kc = np.asarray(k_cache, dtype=np.float32).reshape(NBLOCKS * BS, D).copy()
    vc = np.asarray(v_cache, dtype=np.float32).reshape(NBLOCKS * BS, D).copy()
    slot = np.asarray(slot_mapping, dtype=np.int64)
    keep = slot >= 0
    kc[slot[keep]] = np.asarray(k, dtype=np.float32).reshape(B, D)[keep]
    vc[slot[keep]] = np.asarray(v, dtype=np.float32).reshape(B, D)[keep]
    kc = kc.reshape(NBLOCKS, BS, KVH, DH)
    vc = vc.reshape(NBLOCKS, BS, KVH, DH)

    bt = np.asarray(block_tables, dtype=np.int64)
    ctx = np.asarray(context_lens, dtype=np.int64)
    qf = np.asarray(q, dtype=np.float32)

    # Valid-skip: requests are relabeled (the device program is symmetric in
    # request index) so each gather pair holds one long-context and one
    # short-context request; the short one's invalid block-table tail becomes
    # -1 indices, which the gather ucode skips entirely.
    nblk = np.minimum((ctx + BS - 1) // BS, MBS)  # valid blocks per request
    order = np.argsort(-nblk, kind="stable")     # ranks, descending nblk
    perm = np.empty(B, dtype=np.int64)           # virtual v -> original req
    for p in range(GROUPS):
        perm[2 * p] = order[p]                   # large
        perm[2 * p + 1] = order[B - 1 - p]       # small

    # idx tile: per pair, 128 ids of the large request then the small
    # request's valid prefix + -1 tail; wrapped i = s*16 + p -> [p, s].
    idx = np.zeros((128, GROUPS * (NIDX // 16)), dtype=np.int16)
    nv = np.zeros((1, GROUPS), dtype=np.int32)
    for g in range(GROUPS):
        big, small = perm[2 * g], perm[2 * g + 1]
        ids = np.full(NIDX, -1, dtype=np.int16)
        ids[:MBS] = bt[big].astype(np.int16)
        ns = int(nblk[small])
        ids[MBS:MBS + ns] = bt[small, :ns].astype(np.int16)
        nv[0, g] = MBS + ns
        w = ids.reshape(NIDX // 16, 16).T  # [16, 16]
        idx[:, g * (NIDX // 16):(g + 1) * (NIDX // 16)] = np.tile(w, (8, 1))

    # mask [128, 2048]: row 4v+h, col sl*128 + j -> position j*16+sl of
    # virtual request v (original perm[v])
    j = np.arange(MBS)
    sl = np.arange(BS)
    pos = (j[None, :] * BS + sl[:, None]).reshape(S)  # col -> seq position
    valid = pos[None, :] < ctx[perm][:, None]  # [B, S] virtual order
    mask_rows = np.where(valid, 0.0, NEG).astype(np.float32)  # [B, S]
    mask = np.repeat(mask_rows, NH, axis=0)  # [128, S]

    ident = np.eye(128, dtype=np.float32).astype(BF16)

    in_maps = []
    for kh in range(KVH):
        k_pool = np.ascontiguousarray(
            kc[:, :, kh, :].reshape(NBLOCKS, BS * DH)).astype(BF16)
        v_pool = np.ascontiguousarray(
            vc[:, :, kh, :].reshape(NBLOCKS, BS * DH)).astype(BF16)
        qpad = np.zeros((DH, B * 128), dtype=np.float32)
        for v in range(B):
            # stationary cols 4v..4v+3 of slice v hold q^T * SCALE
            qpad[:, v * 128 + NH * v: v * 128 + NH * v + NH] = (
                qf[perm[v], NH * kh: NH * (kh + 1), :].T * SCALE
            )
        in_maps.append({
            "k_pool": k_pool,
            "v_pool": v_pool,
            "qpad": qpad.astype(BF16),
            "mask": mask,
            "idx": idx,
            "nv": nv,
            "ident": ident,
        })
    return in_maps, perm


def kernel(q, k, v, k_cache, v_cache, slot_mapping, block_tables, context_lens):
    from concourse.bass_utils import run_bass_kernel_spmd

    nc = build_core_program()
    in_maps, perm = _host_inputs(
        q, k, v, k_cache, v_cache, slot_mapping, block_tables, context_lens
    )
    core_ids = list(range(KVH))
    res = run_bass_kernel_spmd(
        nc, in_maps, core_ids,
        trace=bool(int(os.environ.get("KERNEL_TRACE", "0"))),
        tmpdir=os.environ.get("KERNEL_TMPDIR") or None,
    )
    kernel.last_results = res
    outs = res.results
    full = np.empty((B, KVH * NH, DH), dtype=np.float32)
    for kh in range(KVH):
        oc = np.asarray(outs[kh]["out"], dtype=np.float32).reshape(B, NH, DH)
        full[perm, NH * kh: NH * (kh + 1), :] = oc  # unpermute virtual order
    return full


# revision 42
# speedup vs baseline: 1.0846x; 1.0846x over previous
"""Paged-attention decode kernel for 8 TRN2 NeuronCores.

Sharding: tensor-parallel over the 8 KV heads (one per core). Each core holds
its own 128-wide slice of the paged KV cache (converted to bf16), computes the
4 GQA query heads of its group for all 32 requests, and writes a [128, 128]
output block ([32 req x 4 heads, 128 dim]). The host applies the KV-cache
scatter update, builds per-core pools/indices/masks, and concatenates the 8
per-core outputs into the full [32, 32, 128] result. No collectives needed.

Device algorithm per core:
  - dma_gather(transpose=True) over block rows of the bf16 K pool yields the
    K^T layout [d=128, slot, block] directly (one gather per 4-request group).
  - QK matmuls use a zero-padded stationary q so request b's scores land on
    PSUM partitions 4b..4b+3; all 32 requests accumulate into one
    [128, 2048] PSUM scores region ([req*4+head, slot*128+block]).
  - Batched masked softmax over the full [128, 2048] region (mask from
    context_lens, built host-side).
  - 16 PE transposes produce p^T; dma_gather(transpose=False) yields V blocks
    [block, slot*128+d]; PV matmuls contract over blocks per (request, slot),
    accumulating [4, 128] per request in PSUM.
"""

import os
import sys

import numpy as np
import ml_dtypes

if "/opt/trn_rl_repo" not in sys.path:
    sys.path.insert(0, "/opt/trn_rl_repo")

import concourse.bacc as bacc
import concourse.bass as bass
import concourse.mybir as mybir
import concourse.tile as tile

BF16 = ml_dtypes.bfloat16

SCALE = 0.08838834764831845  # 1/sqrt(128)
B = 32               # requests
KVH = 8              # kv heads == cores
NH = 4               # q heads per kv head (GQA group)
DH = 128             # head dim
BS = 16              # tokens per cache block
NBLOCKS = 4096       # pool blocks
MBS = 128            # max blocks per sequence
S = MBS * BS         # 2048 max context
GROUPS = 8           # request groups per core
GR = B // GROUPS     # 4 requests per group
NIDX = GR * MBS      # 512 gathered blocks per group
NEG = -1.0e30


NQUEUES = 1
DETECT_RACES = True  # sim-only; the SWDGE-prep sem rewrite confuses the
                     # race detector's semaphore epoch accounting


def build_core_program():
    """Build the single-core Bass program (same on all 8 cores)."""
    nc = bacc.Bacc(
        "TRN2", target_bir_lowering=False, num_swdge_queues=NQUEUES,
        detect_race_conditions=DETECT_RACES,
    )
    f32 = mybir.dt.float32
    bf16 = mybir.dt.bfloat16
    i16 = mybir.dt.int16

    k_pool = nc.dram_tensor("k_pool", [NBLOCKS, BS * DH], bf16, kind="ExternalInput")
    v_pool = nc.dram_tensor("v_pool", [NBLOCKS, BS * DH], bf16, kind="ExternalInput")
    qpad = nc.dram_tensor("qpad", [DH, B * 128], bf16, kind="ExternalInput")
    maskd = nc.dram_tensor("mask", [128, S], f32, kind="ExternalInput")
    idxd = nc.dram_tensor("idx", [128, GROUPS * (NIDX // 16)], i16, kind="ExternalInput")
    ident = nc.dram_tensor("ident", [128, 128], bf16, kind="ExternalInput")
    out = nc.dram_tensor("out", [128, DH], f32, kind="ExternalOutput")

    Exp = mybir.ActivationFunctionType.Exp
    ICOLS = NIDX // 16  # 32 idx columns per group

    with tile.TileContext(nc) as tc:
        with (
            tc.tile_pool(name="const", bufs=1) as cpool,
            tc.tile_pool(name="soft", bufs=1) as spool,
            tc.tile_pool(name="kt", bufs=2) as ktpool,
            tc.tile_pool(name="vv", bufs=5) as vpool,
            tc.tile_pool(name="outs", bufs=4) as ospool,
        ):
            # preload the gather ucode library so its ~13us fetch overlaps
            # the input DMAs instead of stalling the first gather
            from concourse.library_config import mlp as _mlp_lib
            nc.gpsimd.load_library(_mlp_lib)

            qpad_sb = cpool.tile([DH, B * 128], bf16)
            mask_sb = cpool.tile([128, S], f32)
            idx_sb = cpool.tile([128, GROUPS * ICOLS], i16)
            id_sb = cpool.tile([128, 128], bf16)
            nc.sync.dma_start(idx_sb[:], idxd[:])
            nc.sync.dma_start(qpad_sb[:], qpad[:])
            nc.sync.dma_start(mask_sb[:], maskd[:])
            nc.sync.dma_start(id_sb[:], ident[:])

            # one shared register for num_idxs: a fresh to_reg per gather
            # would add a MOVE whose WAR dep serializes gathers on the
            # previous gather's DMA completion
            nidx_reg = nc.gpsimd.to_reg(NIDX)

            s_sb = spool.tile([128, S], f32)
            p_sb = spool.tile([128, S], bf16)
            p2_sb = spool.tile([128, S], bf16)
            pt_sb = spool.tile([128, S], bf16)
            mx = spool.tile([128, 1], f32)
            negm = spool.tile([128, 1], f32)
            sums = spool.tile([128, 1], f32)
            recip = spool.tile([128, 1], f32)

            # ---- Phase B: K gathers + QK matmuls into one PSUM scores region
            NMM = (BS + 3) // 4  # N<=512 chunks of up-to-4 slots each
            with tc.tile_pool(name="pscore", bufs=1, space="PSUM") as pspool:
                scores = pspool.tile([128, S], f32)
                for g in range(GROUPS):
                    kt = ktpool.tile([128, BS, NIDX], bf16, tag="kt")
                    nc.gpsimd.dma_gather(
                        kt[:],
                        k_pool[:],
                        idx_sb[:, g * ICOLS:(g + 1) * ICOLS],
                        NIDX,
                        nidx_reg,
                        BS * DH,
                        transpose=True,
                        queue_num=g % NQUEUES,
                    )
                    for r in range(GR):
                        b = GR * g + r
                        for mm in range(NMM):
                            nsl = min(4, BS - mm * 4)
                            nc.tensor.matmul(
                                scores[:, mm * 512: mm * 512 + nsl * 128],
                                lhsT=qpad_sb[:, b * 128:(b + 1) * 128],
                                rhs=kt[:, mm * 4: mm * 4 + nsl, r * 128:(r + 1) * 128],
                                start=(b == 0),
                                stop=(b == B - 1),
                            )

                # ---- Phase C: batched masked softmax
                nc.vector.tensor_tensor(
                    out=s_sb[:], in0=scores[:], in1=mask_sb[:], op=mybir.AluOpType.add
                )
            nc.vector.reduce_max(mx[:], s_sb[:], axis=mybir.AxisListType.X)
            nc.scalar.mul(negm[:], mx[:], -1.0)
            nc.scalar.activation(
                p_sb[:], s_sb[:], Exp, bias=negm[:, 0:1], scale=1.0,
                accum_out=sums[:, 0:1],
            )
            nc.vector.reciprocal(recip[:], sums[:])
            nc.vector.tensor_scalar_mul(p2_sb[:], p_sb[:], recip[:, 0:1])

            # ---- Phase D: p^T via PE transposes
            with tc.tile_pool(name="ptr", bufs=2, space="PSUM") as tppool:
                for cc in range(BS):
                    tp = tppool.tile([128, 128], bf16, tag="tp")
                    nc.tensor.transpose(tp[:], p2_sb[:, cc * 128:(cc + 1) * 128], id_sb[:])
                    if cc % 2 == 0:
                        nc.vector.tensor_copy(pt_sb[:, cc * 128:(cc + 1) * 128], tp[:])
                    else:
                        nc.scalar.copy(pt_sb[:, cc * 128:(cc + 1) * 128], tp[:])

            # ---- Phase E: V gathers + PV matmuls
            with tc.tile_pool(name="pout", bufs=4, space="PSUM") as popool:
                for g in range(GROUPS):
                    vt = vpool.tile([128, GR, BS * DH], bf16, tag="vt")
                    nc.gpsimd.dma_gather(
                        vt[:],
                        v_pool[:],
                        idx_sb[:, g * ICOLS:(g + 1) * ICOLS],
                        NIDX,
                        nidx_reg,
                        BS * DH,
                        transpose=False,
                        queue_num=g % NQUEUES,
                    )
                    for r in range(GR):
                        b = GR * g + r
                        po = popool.tile([NH, DH], mybir.dt.float32, tag="po")
                        for sl in range(BS):
                            nc.tensor.matmul(
                                po[:],
                                lhsT=pt_sb[:, sl * 128 + NH * b: sl * 128 + NH * b + NH],
                                rhs=vt[:, r, sl * DH:(sl + 1) * DH],
                                start=(sl == 0),
                                stop=(sl == BS - 1),
                            )
                        os_t = ospool.tile([NH, DH], mybir.dt.float32, tag="os")
                        nc.vector.tensor_copy(os_t[:], po[:])
                        nc.sync.dma_start(out[NH * b: NH * b + NH, :], os_t[:])

    nc.compile()
    _fix_prep_completion_sems(nc)
    return nc


def _fix_prep_completion_sems(nc):
    """Tile gates consumers of a prepare_only SWDGE gather on its DMASW lane
    semaphore, but the DMA-completion sem baked into the descriptors stays the
    caller-provided one — the lane sem would never fire. Rewrite each prep's
    on_update[0] to the lane sem of its scheduled DMASW proc."""
    from concourse.tile_sem_assignment import PROC_NAME_TO_IDX

    idx_to_lane = {v: k for k, v in PROC_NAME_TO_IDX.items() if "DMASW" in k}
    # sem ant_name -> (id, name) for tile-created DMASW sems
    sems = {}
    for bb in nc.main_func.blocks:
        for ins in bb.instructions:
            si = ins.sync_info
            if not si:
                continue
            for ev in list(si.on_wait or []) + list(si.on_update or []):
                name = getattr(ev, "ant_name", None)
                if name and name.startswith("DMASW"):
                    sems[name.split("_")[0]] = (ev.id, name)
    for bb in nc.main_func.blocks:
        for ins in bb.instructions:
            if type(ins).__name__ != "InstDMAGatherAnt" or ins.gen_mode != 1:
                continue
            proc = ins.bass_scheduled_proc
            lane = idx_to_lane.get(proc)
            assert lane is not None, f"prep {ins.name} not on a DMASW lane: {proc}"
            assert lane in sems, f"no tile sem found for {lane}"
            sid, sname = sems[lane]
            upd = ins.sync_info.on_update[0]
            assert upd.ant_name.startswith("kdma"), upd.ant_name
            upd.id = sid
            upd.ant_name = sname


def _host_inputs(q, k, v, k_cache, v_cache, slot_mapping, block_tables, context_lens):
    """Apply the scatter update and build per-core input dicts."""
    D = KVH * DH
    kc = np.asarray(k_cache, dtype=np.float32).reshape(NBLOCKS * BS, D).copy()
    vc = np.asarray(v_cache, dtype=np.float32).reshape(NBLOCKS * BS, D).copy()
    slot = np.asarray(slot_mapping, dtype=np.int64)
    keep = slot >= 0
    kc[slot[keep]] = np.asarray(k, dtype=np.float32).reshape(B, D)[keep]
    vc[slot[keep]] = np.asarray(v, dtype=np.float32).reshape(B, D)[keep]
    kc = kc.reshape(NBLOCKS, BS, KVH, DH)
    vc = vc.reshape(NBLOCKS, BS, KVH, DH)

    bt = np.asarray(block_tables, dtype=np.int64)
    ctx = np.asarray(context_lens, dtype=np.int64)
    qf = np.asarray(q, dtype=np.float32)

    perm = np.arange(B, dtype=np.int64)  # identity relabeling

    # idx tile: per group g, 512 block ids (requests 4g..4g+3 concatenated),
    # wrapped: linear i = s*16 + p -> [p, s]; replicated to 128 partitions.
    ic = NIDX // 16
    idx = np.zeros((128, GROUPS * ic), dtype=np.int16)
    for g in range(GROUPS):
        ids = bt[GR * g:GR * (g + 1)].reshape(NIDX).astype(np.int16)
        w = ids.reshape(ic, 16).T
        idx[:, g * ic:(g + 1) * ic] = np.tile(w, (8, 1))

    # mask [128, 2048]: row 4b+h, col sl*128 + j -> position j*16+sl
    j = np.arange(MBS)
    sl = np.arange(BS)
    pos = (j[None, :] * BS + sl[:, None]).reshape(S)  # col -> seq position
    valid = pos[None, :] < ctx[:, None]  # [B, S]
    mask_rows = np.where(valid, 0.0, NEG).astype(np.float32)  # [B, S]
    mask = np.repeat(mask_rows, NH, axis=0)  # [128, S]

    ident = np.eye(128, dtype=np.float32).astype(BF16)

    in_maps = []
    for kh in range(KVH):
        k_pool = np.ascontiguousarray(
            kc[:, :, kh, :].reshape(NBLOCKS, BS * DH)).astype(BF16)
        v_pool = np.ascontiguousarray(
            vc[:, :, kh, :].reshape(NBLOCKS, BS * DH)).astype(BF16)
        qpad = np.zeros((DH, B * 128), dtype=np.float32)
        for v in range(B):
            # stationary cols 4v..4v+3 of slice v hold q^T * SCALE
            qpad[:, v * 128 + NH * v: v * 128 + NH * v + NH] = (
                qf[perm[v], NH * kh: NH * (kh + 1), :].T * SCALE
            )
        in_maps.append({
            "k_pool": k_pool,
            "v_pool": v_pool,
            "qpad": qpad.astype(BF16),
            "mask": mask,
            "idx": idx,
            "ident": ident,
        })
    return in_maps, perm


def kernel(q, k, v, k_cache, v_cache, slot_mapping, block_tables, context_lens):
    from concourse.bass_utils import run_bass_kernel_spmd

    nc = build_core_program()
    in_maps, perm = _host_inputs(
        q, k, v, k_cache, v_cache, slot_mapping, block_tables, context_lens
    )
    core_ids = list(range(KVH))
    res = run_bass_kernel_spmd(
        nc, in_maps, core_ids,
        trace=bool(int(os.environ.get("KERNEL_TRACE", "0"))),
        tmpdir=os.environ.get("KERNEL_TMPDIR") or None,
    )
    kernel.last_results = res
    outs = res.results
    full = np.empty((B, KVH * NH, DH), dtype=np.float32)
    for kh in range(KVH):
        oc = np.asarray(outs[kh]["out"], dtype=np.float32).reshape(B, NH, DH)
        full[perm, NH * kh: NH * (kh + 1), :] = oc  # unpermute virtual order
    return full


# revision 43
# speedup vs baseline: 1.1541x; 1.0641x over previous
"""Paged-attention decode kernel for 8 TRN2 NeuronCores.

Sharding: tensor-parallel over the 8 KV heads (one per core). Each core holds
its own 128-wide slice of the paged KV cache (converted to bf16), computes the
4 GQA query heads of its group for all 32 requests, and writes a [128, 128]
output block ([32 req x 4 heads, 128 dim]). The host applies the KV-cache
scatter update, builds per-core pools/indices/masks, and concatenates the 8
per-core outputs into the full [32, 32, 128] result. No collectives needed.

Device algorithm per core:
  - dma_gather(transpose=True) over block rows of the bf16 K pool yields the
    K^T layout [d=128, slot, block] directly (one gather per 4-request group).
  - QK matmuls use a zero-padded stationary q so request b's scores land on
    PSUM partitions 4b..4b+3; all 32 requests accumulate into one
    [128, 2048] PSUM scores region ([req*4+head, slot*128+block]).
  - Batched masked softmax over the full [128, 2048] region (mask from
    context_lens, built host-side).
  - 16 PE transposes produce p^T; dma_gather(transpose=False) yields V blocks
    [block, slot*128+d]; PV matmuls contract over blocks per (request, slot),
    accumulating [4, 128] per request in PSUM.
"""

import os
import sys

import numpy as np
import ml_dtypes

if "/opt/trn_rl_repo" not in sys.path:
    sys.path.insert(0, "/opt/trn_rl_repo")

import concourse.bacc as bacc
import concourse.bass as bass
import concourse.mybir as mybir
import concourse.tile as tile

BF16 = ml_dtypes.bfloat16

SCALE = 0.08838834764831845  # 1/sqrt(128)
B = 32               # requests
KVH = 8              # kv heads == cores
NH = 4               # q heads per kv head (GQA group)
DH = 128             # head dim
BS = 16              # tokens per cache block
NBLOCKS = 4096       # pool blocks
MBS = 128            # max blocks per sequence
S = MBS * BS         # 2048 max context
GROUPS = 8           # request groups per core
GR = B // GROUPS     # 4 requests per group
NIDX = GR * MBS      # 512 gathered blocks per group
NEG = -1.0e30


NQUEUES = 1
DETECT_RACES = True  # sim-only; the SWDGE-prep sem rewrite confuses the
                     # race detector's semaphore epoch accounting


def build_core_program():
    """Build the single-core Bass program (same on all 8 cores)."""
    nc = bacc.Bacc(
        "TRN2", target_bir_lowering=False, num_swdge_queues=NQUEUES,
        detect_race_conditions=DETECT_RACES,
    )
    f32 = mybir.dt.float32
    bf16 = mybir.dt.bfloat16
    i16 = mybir.dt.int16

    k_pool = nc.dram_tensor("k_pool", [NBLOCKS, BS * DH], bf16, kind="ExternalInput")
    v_pool = nc.dram_tensor("v_pool", [NBLOCKS, BS * DH], bf16, kind="ExternalInput")
    qpad = nc.dram_tensor("qpad", [DH, B * 128], bf16, kind="ExternalInput")
    maskd = nc.dram_tensor("mask", [128, S], f32, kind="ExternalInput")
    idxd = nc.dram_tensor("idx", [128, GROUPS * (NIDX // 16)], i16, kind="ExternalInput")
    ident = nc.dram_tensor("ident", [128, 128], bf16, kind="ExternalInput")
    out = nc.dram_tensor("out", [128, DH], f32, kind="ExternalOutput")

    Exp = mybir.ActivationFunctionType.Exp
    ICOLS = NIDX // 16  # 32 idx columns per group

    with tile.TileContext(nc) as tc:
        with (
            tc.tile_pool(name="const", bufs=1) as cpool,
            tc.tile_pool(name="soft", bufs=1) as spool,
            tc.tile_pool(name="kt", bufs=3) as ktpool,
            tc.tile_pool(name="vv", bufs=3) as vpool,
            tc.tile_pool(name="outs", bufs=4) as ospool,
        ):
            # preload the gather ucode library so its ~13us fetch overlaps
            # the input DMAs instead of stalling the first gather
            from concourse.library_config import mlp as _mlp_lib
            nc.gpsimd.load_library(_mlp_lib)

            qpad_sb = cpool.tile([DH, B * 128], bf16)
            mask_sb = cpool.tile([128, S], f32)
            idx_sb = cpool.tile([128, GROUPS * ICOLS], i16)
            id_sb = cpool.tile([128, 128], bf16)
            nc.sync.dma_start(idx_sb[:], idxd[:])
            nc.sync.dma_start(qpad_sb[:], qpad[:])
            nc.sync.dma_start(mask_sb[:], maskd[:])
            nc.sync.dma_start(id_sb[:], ident[:])

            # one shared register for num_idxs: a fresh to_reg per gather
            # would add a MOVE whose WAR dep serializes gathers on the
            # previous gather's DMA completion
            nidx_reg = nc.gpsimd.to_reg(NIDX)

            s_sb = spool.tile([128, S], f32)
            p_sb = spool.tile([128, S], bf16)
            p2_sb = spool.tile([128, S], bf16)
            pt_sb = spool.tile([128, S], bf16)
            mx = spool.tile([128, 1], f32)
            negm = spool.tile([128, 1], f32)
            sums = spool.tile([128, 1], f32)
            recip = spool.tile([128, 1], f32)

            # ---- Phase B: K gathers + QK matmuls into one PSUM scores region
            NMM = (BS + 3) // 4  # N<=512 chunks of up-to-4 slots each
            with tc.tile_pool(name="pscore", bufs=1, space="PSUM") as pspool:
                scores = pspool.tile([128, S], f32)
                for g in range(GROUPS):
                    kt = ktpool.tile([128, BS, NIDX], bf16, tag="kt")
                    nc.gpsimd.dma_gather(
                        kt[:],
                        k_pool[:],
                        idx_sb[:, g * ICOLS:(g + 1) * ICOLS],
                        NIDX,
                        nidx_reg,
                        BS * DH,
                        transpose=True,
                        queue_num=g % NQUEUES,
                    )
                    for r in range(GR):
                        b = GR * g + r
                        for mm in range(NMM):
                            nsl = min(4, BS - mm * 4)
                            nc.tensor.matmul(
                                scores[:, mm * 512: mm * 512 + nsl * 128],
                                lhsT=qpad_sb[:, b * 128:(b + 1) * 128],
                                rhs=kt[:, mm * 4: mm * 4 + nsl, r * 128:(r + 1) * 128],
                                start=(b == 0),
                                stop=(b == B - 1),
                            )

                # ---- Phase C: batched masked softmax
                nc.vector.tensor_tensor(
                    out=s_sb[:], in0=scores[:], in1=mask_sb[:], op=mybir.AluOpType.add
                )
            nc.vector.reduce_max(mx[:], s_sb[:], axis=mybir.AxisListType.X)
            nc.scalar.mul(negm[:], mx[:], -1.0)
            nc.scalar.activation(
                p_sb[:], s_sb[:], Exp, bias=negm[:, 0:1], scale=1.0,
                accum_out=sums[:, 0:1],
            )
            nc.vector.reciprocal(recip[:], sums[:])
            nc.vector.tensor_scalar_mul(p2_sb[:], p_sb[:], recip[:, 0:1])

            # ---- Phase D: p^T via PE transposes
            with tc.tile_pool(name="ptr", bufs=2, space="PSUM") as tppool:
                for cc in range(BS):
                    tp = tppool.tile([128, 128], bf16, tag="tp")
                    nc.tensor.transpose(tp[:], p2_sb[:, cc * 128:(cc + 1) * 128], id_sb[:])
                    if cc % 2 == 0:
                        nc.vector.tensor_copy(pt_sb[:, cc * 128:(cc + 1) * 128], tp[:])
                    else:
                        nc.scalar.copy(pt_sb[:, cc * 128:(cc + 1) * 128], tp[:])

            # ---- Phase E: V gathers + PV matmuls
            with tc.tile_pool(name="pout", bufs=4, space="PSUM") as popool:
                for g in range(GROUPS):
                    vt = vpool.tile([128, GR, BS * DH], bf16, tag="vt")
                    nc.gpsimd.dma_gather(
                        vt[:],
                        v_pool[:],
                        idx_sb[:, g * ICOLS:(g + 1) * ICOLS],
                        NIDX,
                        nidx_reg,
                        BS * DH,
                        transpose=False,
                        queue_num=g % NQUEUES,
                    )
                    for r in range(GR):
                        b = GR * g + r
                        po = popool.tile([NH, DH], mybir.dt.float32, tag="po")
                        for sl in range(BS):
                            nc.tensor.matmul(
                                po[:],
                                lhsT=pt_sb[:, sl * 128 + NH * b: sl * 128 + NH * b + NH],
                                rhs=vt[:, r, sl * DH:(sl + 1) * DH],
                                start=(sl == 0),
                                stop=(sl == BS - 1),
                            )
                        os_t = ospool.tile([NH, DH], mybir.dt.float32, tag="os")
                        nc.vector.tensor_copy(os_t[:], po[:])
                        nc.sync.dma_start(out[NH * b: NH * b + NH, :], os_t[:])

    nc.compile()
    _fix_prep_completion_sems(nc)
    return nc


def _fix_prep_completion_sems(nc):
    """Tile gates consumers of a prepare_only SWDGE gather on its DMASW lane
    semaphore, but the DMA-completion sem baked into the descriptors stays the
    caller-provided one — the lane sem would never fire. Rewrite each prep's
    on_update[0] to the lane sem of its scheduled DMASW proc."""
    from concourse.tile_sem_assignment import PROC_NAME_TO_IDX

    idx_to_lane = {v: k for k, v in PROC_NAME_TO_IDX.items() if "DMASW" in k}
    # sem ant_name -> (id, name) for tile-created DMASW sems
    sems = {}
    for bb in nc.main_func.blocks:
        for ins in bb.instructions:
            si = ins.sync_info
            if not si:
                continue
            for ev in list(si.on_wait or []) + list(si.on_update or []):
                name = getattr(ev, "ant_name", None)
                if name and name.startswith("DMASW"):
                    sems[name.split("_")[0]] = (ev.id, name)
    for bb in nc.main_func.blocks:
        for ins in bb.instructions:
            if type(ins).__name__ != "InstDMAGatherAnt" or ins.gen_mode != 1:
                continue
            proc = ins.bass_scheduled_proc
            lane = idx_to_lane.get(proc)
            assert lane is not None, f"prep {ins.name} not on a DMASW lane: {proc}"
            assert lane in sems, f"no tile sem found for {lane}"
            sid, sname = sems[lane]
            upd = ins.sync_info.on_update[0]
            assert upd.ant_name.startswith("kdma"), upd.ant_name
            upd.id = sid
            upd.ant_name = sname


def _host_inputs(q, k, v, k_cache, v_cache, slot_mapping, block_tables, context_lens):
    """Apply the scatter update and build per-core input dicts."""
    D = KVH * DH
    kc = np.asarray(k_cache, dtype=np.float32).reshape(NBLOCKS * BS, D).copy()
    vc = np.asarray(v_cache, dtype=np.float32).reshape(NBLOCKS * BS, D).copy()
    slot = np.asarray(slot_mapping, dtype=np.int64)
    keep = slot >= 0
    kc[slot[keep]] = np.asarray(k, dtype=np.float32).reshape(B, D)[keep]
    vc[slot[keep]] = np.asarray(v, dtype=np.float32).reshape(B, D)[keep]
    kc = kc.reshape(NBLOCKS, BS, KVH, DH)
    vc = vc.reshape(NBLOCKS, BS, KVH, DH)

    bt = np.asarray(block_tables, dtype=np.int64)
    ctx = np.asarray(context_lens, dtype=np.int64)
    qf = np.asarray(q, dtype=np.float32)

    perm = np.arange(B, dtype=np.int64)  # identity relabeling

    # idx tile: per group g, 512 block ids (requests 4g..4g+3 concatenated),
    # wrapped: linear i = s*16 + p -> [p, s]; replicated to 128 partitions.
    ic = NIDX // 16
    idx = np.zeros((128, GROUPS * ic), dtype=np.int16)
    for g in range(GROUPS):
        ids = bt[GR * g:GR * (g + 1)].reshape(NIDX).astype(np.int16)
        w = ids.reshape(ic, 16).T
        idx[:, g * ic:(g + 1) * ic] = np.tile(w, (8, 1))

    # mask [128, 2048]: row 4b+h, col sl*128 + j -> position j*16+sl
    j = np.arange(MBS)
    sl = np.arange(BS)
    pos = (j[None, :] * BS + sl[:, None]).reshape(S)  # col -> seq position
    valid = pos[None, :] < ctx[:, None]  # [B, S]
    mask_rows = np.where(valid, 0.0, NEG).astype(np.float32)  # [B, S]
    mask = np.repeat(mask_rows, NH, axis=0)  # [128, S]

    ident = np.eye(128, dtype=np.float32).astype(BF16)

    in_maps = []
    for kh in range(KVH):
        k_pool = np.ascontiguousarray(
            kc[:, :, kh, :].reshape(NBLOCKS, BS * DH)).astype(BF16)
        v_pool = np.ascontiguousarray(
            vc[:, :, kh, :].reshape(NBLOCKS, BS * DH)).astype(BF16)
        qpad = np.zeros((DH, B * 128), dtype=np.float32)
        for v in range(B):
            # stationary cols 4v..4v+3 of slice v hold q^T * SCALE
            qpad[:, v * 128 + NH * v: v * 128 + NH * v + NH] = (
                qf[perm[v], NH * kh: NH * (kh + 1), :].T * SCALE
            )
        in_maps.append({
            "k_pool": k_pool,
            "v_pool": v_pool,
            "qpad": qpad.astype(BF16),
            "mask": mask,
            "idx": idx,
            "ident": ident,
        })
    return in_maps, perm


def kernel(q, k, v, k_cache, v_cache, slot_mapping, block_tables, context_lens):
    from concourse.bass_utils import run_bass_kernel_spmd

    nc = build_core_program()
    in_maps, perm = _host_inputs(
        q, k, v, k_cache, v_cache, slot_mapping, block_tables, context_lens
    )
    core_ids = list(range(KVH))
    res = run_bass_kernel_spmd(
        nc, in_maps, core_ids,
        trace=bool(int(os.environ.get("KERNEL_TRACE", "0"))),
        tmpdir=os.environ.get("KERNEL_TMPDIR") or None,
    )
    kernel.last_results = res
    outs = res.results
    full = np.empty((B, KVH * NH, DH), dtype=np.float32)
    for kh in range(KVH):
        oc = np.asarray(outs[kh]["out"], dtype=np.float32).reshape(B, NH, DH)
        full[perm, NH * kh: NH * (kh + 1), :] = oc  # unpermute virtual order
    return full


# revision 44
# speedup vs baseline: 1.2178x; 1.0552x over previous
"""Paged-attention decode kernel for 8 TRN2 NeuronCores.

Sharding: tensor-parallel over the 8 KV heads (one per core). Each core holds
its own 128-wide slice of the paged KV cache (converted to bf16), computes the
4 GQA query heads of its group for all 32 requests, and writes a [128, 128]
output block ([32 req x 4 heads, 128 dim]). The host applies the KV-cache
scatter update, builds per-core pools/indices/masks, and concatenates the 8
per-core outputs into the full [32, 32, 128] result. No collectives needed.

Device algorithm per core:
  - dma_gather(transpose=True) over block rows of the bf16 K pool yields the
    K^T layout [d=128, slot, block] directly (one gather per 4-request group).
  - QK matmuls use a zero-padded stationary q so request b's scores land on
    PSUM partitions 4b..4b+3; all 32 requests accumulate into one
    [128, 2048] PSUM scores region ([req*4+head, slot*128+block]).
  - Batched masked softmax over the full [128, 2048] region (mask from
    context_lens, built host-side).
  - 16 PE transposes produce p^T; dma_gather(transpose=False) yields V blocks
    [block, slot*128+d]; PV matmuls contract over blocks per (request, slot),
    accumulating [4, 128] per request in PSUM.
"""

import os
import sys

import numpy as np
import ml_dtypes

if "/opt/trn_rl_repo" not in sys.path:
    sys.path.insert(0, "/opt/trn_rl_repo")

import concourse.bacc as bacc
import concourse.bass as bass
import concourse.mybir as mybir
import concourse.tile as tile

BF16 = ml_dtypes.bfloat16

SCALE = 0.08838834764831845  # 1/sqrt(128)
B = 32               # requests
KVH = 8              # kv heads == cores
NH = 4               # q heads per kv head (GQA group)
DH = 128             # head dim
BS = 16              # tokens per cache block
NBLOCKS = 4096       # pool blocks
MBS = 128            # max blocks per sequence
S = MBS * BS         # 2048 max context
GROUPS = 8           # request groups per core
GR = B // GROUPS     # 4 requests per group
NIDX = GR * MBS      # 512 gathered blocks per group
NEG = -1.0e30


NQUEUES = 1
DETECT_RACES = True  # sim-only; the SWDGE-prep sem rewrite confuses the
                     # race detector's semaphore epoch accounting


def build_core_program():
    """Build the single-core Bass program (same on all 8 cores)."""
    nc = bacc.Bacc(
        "TRN2", target_bir_lowering=False, num_swdge_queues=NQUEUES,
        detect_race_conditions=DETECT_RACES,
    )
    f32 = mybir.dt.float32
    bf16 = mybir.dt.bfloat16
    i16 = mybir.dt.int16

    k_pool = nc.dram_tensor("k_pool", [NBLOCKS, BS * DH], bf16, kind="ExternalInput")
    v_pool = nc.dram_tensor("v_pool", [NBLOCKS, BS * DH], bf16, kind="ExternalInput")
    qpad = nc.dram_tensor("qpad", [DH, B * 128], bf16, kind="ExternalInput")
    maskd = nc.dram_tensor("mask", [128, S], f32, kind="ExternalInput")
    idxd = nc.dram_tensor("idx", [128, GROUPS * (NIDX // 16)], i16, kind="ExternalInput")
    ident = nc.dram_tensor("ident", [128, 128], bf16, kind="ExternalInput")
    out = nc.dram_tensor("out", [128, DH], f32, kind="ExternalOutput")

    Exp = mybir.ActivationFunctionType.Exp
    ICOLS = NIDX // 16  # 32 idx columns per group

    with tile.TileContext(nc) as tc:
        with (
            tc.tile_pool(name="const", bufs=1) as cpool,
            tc.tile_pool(name="soft", bufs=1) as spool,
            tc.tile_pool(name="kt", bufs=3) as ktpool,
            tc.tile_pool(name="vv", bufs=5) as vpool,
            tc.tile_pool(name="outs", bufs=4) as ospool,
        ):
            # preload the gather ucode library so its ~13us fetch overlaps
            # the input DMAs instead of stalling the first gather
            from concourse.library_config import mlp as _mlp_lib
            nc.gpsimd.load_library(_mlp_lib)

            qpad_sb = cpool.tile([DH, B * 128], bf16)
            mask_sb = cpool.tile([128, S], f32)
            idx_sb = cpool.tile([128, GROUPS * ICOLS], i16)
            id_sb = cpool.tile([128, 128], bf16)
            nc.sync.dma_start(idx_sb[:], idxd[:])
            nc.sync.dma_start(qpad_sb[:], qpad[:])
            nc.sync.dma_start(mask_sb[:], maskd[:])
            nc.sync.dma_start(id_sb[:], ident[:])

            # one shared register for num_idxs: a fresh to_reg per gather
            # would add a MOVE whose WAR dep serializes gathers on the
            # previous gather's DMA completion
            nidx_reg = nc.gpsimd.to_reg(NIDX)

            s_sb = spool.tile([128, S], f32)
            p_sb = spool.tile([128, S], bf16)
            p2_sb = spool.tile([128, S], bf16)
            pt_sb = spool.tile([128, S], bf16)
            mx = spool.tile([128, 1], f32)
            negm = spool.tile([128, 1], f32)
            sums = spool.tile([128, 1], f32)
            recip = spool.tile([128, 1], f32)

            # ---- Phase B: K gathers + QK matmuls into one PSUM scores region
            NMM = (BS + 3) // 4  # N<=512 chunks of up-to-4 slots each
            with tc.tile_pool(name="pscore", bufs=1, space="PSUM") as pspool:
                scores = pspool.tile([128, S], f32)
                for g in range(GROUPS):
                    kt = ktpool.tile([128, BS, NIDX], bf16, tag="kt")
                    nc.gpsimd.dma_gather(
                        kt[:],
                        k_pool[:],
                        idx_sb[:, g * ICOLS:(g + 1) * ICOLS],
                        NIDX,
                        nidx_reg,
                        BS * DH,
                        transpose=True,
                        queue_num=g % NQUEUES,
                    )
                    for r in range(GR):
                        b = GR * g + r
                        for mm in range(NMM):
                            nsl = min(4, BS - mm * 4)
                            nc.tensor.matmul(
                                scores[:, mm * 512: mm * 512 + nsl * 128],
                                lhsT=qpad_sb[:, b * 128:(b + 1) * 128],
                                rhs=kt[:, mm * 4: mm * 4 + nsl, r * 128:(r + 1) * 128],
                                start=(b == 0),
                                stop=(b == B - 1),
                            )

                # ---- Phase C: batched masked softmax
                nc.vector.tensor_tensor(
                    out=s_sb[:], in0=scores[:], in1=mask_sb[:], op=mybir.AluOpType.add
                )
            nc.vector.reduce_max(mx[:], s_sb[:], axis=mybir.AxisListType.X)
            nc.scalar.mul(negm[:], mx[:], -1.0)
            nc.scalar.activation(
                p_sb[:], s_sb[:], Exp, bias=negm[:, 0:1], scale=1.0,
                accum_out=sums[:, 0:1],
            )
            nc.vector.reciprocal(recip[:], sums[:])
            nc.vector.tensor_scalar_mul(p2_sb[:], p_sb[:], recip[:, 0:1])

            # ---- Phase D: p^T via PE transposes
            with tc.tile_pool(name="ptr", bufs=2, space="PSUM") as tppool:
                for cc in range(BS):
                    tp = tppool.tile([128, 128], bf16, tag="tp")
                    nc.tensor.transpose(tp[:], p2_sb[:, cc * 128:(cc + 1) * 128], id_sb[:])
                    if cc % 2 == 0:
                        nc.vector.tensor_copy(pt_sb[:, cc * 128:(cc + 1) * 128], tp[:])
                    else:
                        nc.scalar.copy(pt_sb[:, cc * 128:(cc + 1) * 128], tp[:])

            # ---- Phase E: V gathers + PV matmuls
            with tc.tile_pool(name="pout", bufs=4, space="PSUM") as popool:
                for g in range(GROUPS):
                    vt = vpool.tile([128, GR, BS * DH], bf16, tag="vt")
                    nc.gpsimd.dma_gather(
                        vt[:],
                        v_pool[:],
                        idx_sb[:, g * ICOLS:(g + 1) * ICOLS],
                        NIDX,
                        nidx_reg,
                        BS * DH,
                        transpose=False,
                        queue_num=g % NQUEUES,
                    )
                    for r in range(GR):
                        b = GR * g + r
                        po = popool.tile([NH, DH], mybir.dt.float32, tag="po")
                        for sl in range(BS):
                            nc.tensor.matmul(
                                po[:],
                                lhsT=pt_sb[:, sl * 128 + NH * b: sl * 128 + NH * b + NH],
                                rhs=vt[:, r, sl * DH:(sl + 1) * DH],
                                start=(sl == 0),
                                stop=(sl == BS - 1),
                            )
                        os_t = ospool.tile([NH, DH], mybir.dt.float32, tag="os")
                        nc.vector.tensor_copy(os_t[:], po[:])
                        nc.sync.dma_start(out[NH * b: NH * b + NH, :], os_t[:])

    nc.compile()
    _fix_prep_completion_sems(nc)
    return nc


def _fix_prep_completion_sems(nc):
    """Tile gates consumers of a prepare_only SWDGE gather on its DMASW lane
    semaphore, but the DMA-completion sem baked into the descriptors stays the
    caller-provided one — the lane sem would never fire. Rewrite each prep's
    on_update[0] to the lane sem of its scheduled DMASW proc."""
    from concourse.tile_sem_assignment import PROC_NAME_TO_IDX

    idx_to_lane = {v: k for k, v in PROC_NAME_TO_IDX.items() if "DMASW" in k}
    # sem ant_name -> (id, name) for tile-created DMASW sems
    sems = {}
    for bb in nc.main_func.blocks:
        for ins in bb.instructions:
            si = ins.sync_info
            if not si:
                continue
            for ev in list(si.on_wait or []) + list(si.on_update or []):
                name = getattr(ev, "ant_name", None)
                if name and name.startswith("DMASW"):
                    sems[name.split("_")[0]] = (ev.id, name)
    for bb in nc.main_func.blocks:
        for ins in bb.instructions:
            if type(ins).__name__ != "InstDMAGatherAnt" or ins.gen_mode != 1:
                continue
            proc = ins.bass_scheduled_proc
            lane = idx_to_lane.get(proc)
            assert lane is not None, f"prep {ins.name} not on a DMASW lane: {proc}"
            assert lane in sems, f"no tile sem found for {lane}"
            sid, sname = sems[lane]
            upd = ins.sync_info.on_update[0]
            assert upd.ant_name.startswith("kdma"), upd.ant_name
            upd.id = sid
            upd.ant_name = sname


def _host_inputs(q, k, v, k_cache, v_cache, slot_mapping, block_tables, context_lens):
    """Apply the scatter update and build per-core input dicts."""
    D = KVH * DH
    kc = np.asarray(k_cache, dtype=np.float32).reshape(NBLOCKS * BS, D).copy()
    vc = np.asarray(v_cache, dtype=np.float32).reshape(NBLOCKS * BS, D).copy()
    slot = np.asarray(slot_mapping, dtype=np.int64)
    keep = slot >= 0
    kc[slot[keep]] = np.asarray(k, dtype=np.float32).reshape(B, D)[keep]
    vc[slot[keep]] = np.asarray(v, dtype=np.float32).reshape(B, D)[keep]
    kc = kc.reshape(NBLOCKS, BS, KVH, DH)
    vc = vc.reshape(NBLOCKS, BS, KVH, DH)

    bt = np.asarray(block_tables, dtype=np.int64)
    ctx = np.asarray(context_lens, dtype=np.int64)
    qf = np.asarray(q, dtype=np.float32)

    perm = np.arange(B, dtype=np.int64)  # identity relabeling

    # idx tile: per group g, 512 block ids (requests 4g..4g+3 concatenated),
    # wrapped: linear i = s*16 + p -> [p, s]; replicated to 128 partitions.
    ic = NIDX // 16
    idx = np.zeros((128, GROUPS * ic), dtype=np.int16)
    for g in range(GROUPS):
        ids = bt[GR * g:GR * (g + 1)].reshape(NIDX).astype(np.int16)
        w = ids.reshape(ic, 16).T
        idx[:, g * ic:(g + 1) * ic] = np.tile(w, (8, 1))

    # mask [128, 2048]: row 4b+h, col sl*128 + j -> position j*16+sl
    j = np.arange(MBS)
    sl = np.arange(BS)
    pos = (j[None, :] * BS + sl[:, None]).reshape(S)  # col -> seq position
    valid = pos[None, :] < ctx[:, None]  # [B, S]
    mask_rows = np.where(valid, 0.0, NEG).astype(np.float32)  # [B, S]
    mask = np.repeat(mask_rows, NH, axis=0)  # [128, S]

    ident = np.eye(128, dtype=np.float32).astype(BF16)

    in_maps = []
    for kh in range(KVH):
        k_pool = np.ascontiguousarray(
            kc[:, :, kh, :].reshape(NBLOCKS, BS * DH)).astype(BF16)
        v_pool = np.ascontiguousarray(
            vc[:, :, kh, :].reshape(NBLOCKS, BS * DH)).astype(BF16)
        qpad = np.zeros((DH, B * 128), dtype=np.float32)
        for v in range(B):
            # stationary cols 4v..4v+3 of slice v hold q^T * SCALE
            qpad[:, v * 128 + NH * v: v * 128 + NH * v + NH] = (
                qf[perm[v], NH * kh: NH * (kh + 1), :].T * SCALE
            )
        in_maps.append({
            "k_pool": k_pool,
            "v_pool": v_pool,
            "qpad": qpad.astype(BF16),
            "mask": mask,
            "idx": idx,
            "ident": ident,
        })
    return in_maps, perm


def kernel(q, k, v, k_cache, v_cache, slot_mapping, block_tables, context_lens):
    from concourse.bass_utils import run_bass_kernel_spmd

    nc = build_core_program()
    in_maps, perm = _host_inputs(
        q, k, v, k_cache, v_cache, slot_mapping, block_tables, context_lens
    )
    core_ids = list(range(KVH))
    res = run_bass_kernel_spmd(
        nc, in_maps, core_ids,
        trace=bool(int(os.environ.get("KERNEL_TRACE", "0"))),
        tmpdir=os.environ.get("KERNEL_TMPDIR") or None,
    )
    kernel.last_results = res
    outs = res.results
    full = np.empty((B, KVH * NH, DH), dtype=np.float32)
    for kh in range(KVH):
        oc = np.asarray(outs[kh]["out"], dtype=np.float32).reshape(B, NH, DH)
        full[perm, NH * kh: NH * (kh + 1), :] = oc  # unpermute virtual order
    return full
